# revision 39
# baseline (speedup 1.0000x reference)
"""Causal single-head attention (B=8, S=2048, E=768, H=64) on 8 TRN2 NeuronCores.

Sharding: data-parallel over batch - one batch element per core, no collectives.

v10: fully interleaved projection/attention (no phase barrier) with
dedicated uniform PSUM pools (no mixed-dtype slot sharing, no F16 PSUM):
qk[1] + vt[1] + scores[4] + o_all[2] banks. V transposed to [k, h] layout
by batched xbar DMA (contiguous 3D out). Row sums via exp accum_out,
collected per tile; one batched reciprocal+multiply+store epilogue.
"""

import numpy as np
from contextlib import ExitStack

import concourse.bass as bass
import concourse.tile as tile
from concourse import bacc, mybir
from concourse.bass_utils import run_bass_kernel_spmd

F32 = mybir.dt.float32
F16 = mybir.dt.float16

B, S, E, H = 8, 2048, 768, 64
EC = E // 128          # 6 e-chunks
QT_TILES = S // 128    # 16 query tiles
NEG = -1.0e9
STAG = 3               # AV lags scores by this many tiles


def build_attention_core():
    nc = bacc.Bacc(None, target_bir_lowering=False)
    xt = nc.declare_dram_parameter("xt", (128, 4, EC, 512), F16, isOutput=False)
    wqk = nc.declare_dram_parameter("wqk", (E, 128), F16, isOutput=False)
    wv = nc.declare_dram_parameter("wv", (E, H), F16, isOutput=False)
    mask = nc.declare_dram_parameter("mask", (128, 128), F32, isOutput=False)
    out = nc.declare_dram_parameter("out", (S, H), F32, isOutput=True)

    with ExitStack() as ctx:
        tc = ctx.enter_context(tile.TileContext(nc))
        singles = ctx.enter_context(tc.tile_pool(name="singles", bufs=1))
        sP = ctx.enter_context(tc.tile_pool(name="sP", bufs=4, space="PSUM"))
        oP = ctx.enter_context(tc.tile_pool(name="oP", bufs=1, space="PSUM"))
        pPool = ctx.enter_context(tc.tile_pool(name="pPool", bufs=STAG + 1))
        ptPool = ctx.enter_context(tc.tile_pool(name="ptPool", bufs=STAG + 1))
        stats = ctx.enter_context(tc.tile_pool(name="stats", bufs=2 * (STAG + 2)))

        # ---- all loads on SWDGE, prefetched ----
        wqk_sb = singles.tile([128, EC, 128], F16)
        wv_sb = singles.tile([128, EC, H], F16)
        nc.gpsimd.dma_start(
            out=wqk_sb[:], in_=wqk.rearrange("(c p) m -> p c m", p=128))
        nc.gpsimd.dma_start(
            out=wv_sb[:], in_=wv.rearrange("(c p) m -> p c m", p=128))
        mask_sb = singles.tile([128, 128], F32)
        nc.gpsimd.dma_start(out=mask_sb[:], in_=mask[:])
        xt_sb = singles.tile([128, 4, EC, 512], F16)
        for sb in range(4):
            nc.gpsimd.dma_start(out=xt_sb[:, sb, :, :], in_=xt[:, sb, :, :])

        qt_sb = singles.tile([64, S], F16)
        kt_sb = singles.tile([64, S], F16)
        vt_sb = singles.tile([64, S], F16)
        v_sb = singles.tile([128, QT_TILES, H], F16)

        # all 16 AV accumulators in one persistent 2-bank PSUM region
        o_all = oP.tile([128, QT_TILES, H], F32)
        sums_fin = singles.tile([128, QT_TILES], F32)

        def emit_proj(sb, qkP, vtP):
            """QKV projection for one 512-col s-block."""
            cols = bass.ts(sb, 512)
            qk_ps = qkP.tile([128, 512], F32, tag="qk")
            for c in range(EC):
                nc.tensor.matmul(
                    qk_ps[:], lhsT=wqk_sb[:, c, :], rhs=xt_sb[:, sb, c, :],
                    start=(c == 0), stop=(c == EC - 1),
                )
            vt_ps = vtP.tile([64, 512], F32, tag="vt")
            for c in range(EC):
                nc.tensor.matmul(
                    vt_ps[:], lhsT=wv_sb[:, c, :], rhs=xt_sb[:, sb, c, :],
                    start=(c == 0), stop=(c == EC - 1),
                )
            nc.scalar.copy(qt_sb[:, cols], qk_ps[0:64, :])
            nc.scalar.copy(kt_sb[:, cols], qk_ps[64:128, :])
            nc.scalar.copy(vt_sb[:, cols], vt_ps[:])
            # V -> [k, h] layout: one batched xbar (contiguous 3D out)
            nc.sync.dma_start(
                out=v_sb[:, sb * 4:(sb + 1) * 4, :],
                in_=vt_sb[:, cols], transpose=True,
            )

        live = {}

        def emit_front(i):
            """scores + softmax + transpose for tile i"""
            ki = (i + 1) * 128
            nblk = (ki + 511) // 512
            q_sl = bass.ts(i, 128)
            mx = stats.tile([128, 5], F32, tag="mx")
            negm = stats.tile([128, 1], F32, tag="negm")
            sums = stats.tile([128, 5], F32, tag="sums")

            s_tiles = []
            for b in range(nblk):
                w = min(512, ki - b * 512)
                pool = spools[b % len(spools)]
                s_t = pool.tile([128, 512], F32, tag="s")
                s_tiles.append((s_t, w))
                nc.tensor.matmul(
                    s_t[:, 0:w],
                    lhsT=qt_sb[:, q_sl],
                    rhs=kt_sb[:, b * 512:b * 512 + w],
                    start=True, stop=True,
                )
                if b == nblk - 1:
                    nc.vector.tensor_add(
                        s_t[:, w - 128:w], s_t[:, w - 128:w], mask_sb[:]
                    )
                nc.vector.tensor_reduce(
                    mx[:, b:b + 1], s_t[:, 0:w],
                    axis=mybir.AxisListType.X, op=mybir.AluOpType.max,
                )
            nc.vector.tensor_reduce(
                negm[:], mx[:, 0:nblk],
                axis=mybir.AxisListType.X, op=mybir.AluOpType.max,
                negate=True,
            )

            p_t = pPool.tile([128, S], F16, tag="p")
            for b, (s_t, w) in enumerate(s_tiles):
                nc.scalar.activation(
                    p_t[:, b * 512:b * 512 + w], s_t[:, 0:w],
                    mybir.ActivationFunctionType.Exp,
                    bias=negm[:], scale=1.0, accum_out=sums[:, b:b + 1],
                )
            if nblk > 1:
                nc.vector.tensor_reduce(
                    sums_fin[:, i:i + 1], sums[:, 0:nblk],
                    axis=mybir.AxisListType.X, op=mybir.AluOpType.add,
                )
            else:
                nc.vector.tensor_copy(sums_fin[:, i:i + 1], sums[:, 0:1])

            pt_t = ptPool.tile([128, QT_TILES, 128], F16, tag="pt")
            nc.sync.dma_start(
                out=pt_t[:, 0:i + 1, :], in_=p_t[:, 0:ki], transpose=True,
            )
            live[i] = pt_t

        def emit_back(i):
            """AV accumulating into o_all[:, i, :]"""
            pt_t = live.pop(i)
            for j in range(i + 1):
                nc.tensor.matmul(
                    o_all[:, i, :], lhsT=pt_t[:, j, :], rhs=v_sb[:, j, :],
                    start=(j == 0), stop=(j == i),
                )

        # interleaved emission: projection block sb, then its 4 query tiles
        seen = []
        spools = [sP]
        with (
            tc.tile_pool(name="qkP", bufs=1, space="PSUM") as qkP,
            tc.tile_pool(name="vtP", bufs=1, space="PSUM") as vtP,
        ):
            for sb in range(4):
                emit_proj(sb, qkP, vtP)
                for t in range(sb * 4, sb * 4 + 4):
                    if t >= 12:
                        break
                    if len(seen) >= STAG:
                        emit_back(seen[-STAG])
                    emit_front(t)
                    seen.append(t)
        # proj pools closed: their banks host a second score pool for the
        # big tail tiles
        with tc.tile_pool(name="sP2", bufs=2, space="PSUM") as sP2:
            spools = [sP, sP2]
            for t in [15, 14, 13, 12]:
                if len(seen) >= STAG:
                    emit_back(seen[-STAG])
                emit_front(t)
                seen.append(t)
            for t in seen[-STAG:]:
                emit_back(t)

        # batched epilogue: one reciprocal, one broadcast multiply, one store
        rs_all = stats.tile([128, QT_TILES], F32, tag="rsall")
        nc.vector.reciprocal(rs_all[:], sums_fin[:])
        o_fin = singles.tile([128, QT_TILES, H], F32)
        rs_ap = rs_all[:]
        rs_bcast = bass.AP(
            tensor=rs_ap.tensor,
            offset=rs_ap.offset,
            ap=[rs_ap.ap[0], rs_ap.ap[1], [0, H]],
        )
        nc.vector.tensor_mul(o_fin[:], o_all[:], rs_bcast)
        nc.gpsimd.dma_start(
            out=out.rearrange("(i p) h -> p i h", p=128), in_=o_fin[:]
        )

    nc.finalize()
    return nc


_NC_CACHE = None


def make_in_maps(x, Wq, Wk, Wv):
    scale = np.sqrt(np.float32(E))
    wqk_np = np.concatenate([(Wq * scale).T, Wk.T], axis=1).astype(np.float16)
    wv_np = Wv.T.astype(np.float16)
    mask_np = np.triu(np.full((128, 128), NEG, dtype=np.float32), k=1)
    return [
        {
            "xt": np.ascontiguousarray(
                x[b].T.reshape(6, 128, 4, 512).transpose(1, 2, 0, 3)
            ).astype(np.float16),
            "wqk": wqk_np,
            "wv": wv_np,
            "mask": mask_np,
        }
        for b in range(B)
    ]


def kernel(x: np.ndarray, Wq: np.ndarray, Wk: np.ndarray, Wv: np.ndarray) -> np.ndarray:
    global _NC_CACHE
    assert x.shape == (B, S, E)
    in_maps = make_in_maps(x, Wq, Wk, Wv)

    if _NC_CACHE is None:
        _NC_CACHE = build_attention_core()
    res = run_bass_kernel_spmd(_NC_CACHE, in_maps, core_ids=list(range(B)))
    return np.stack([res.results[b]["out"] for b in range(B)], axis=0)


if __name__ == "__main__":
    rng = np.random.default_rng(0)
    x = rng.standard_normal((B, S, E), dtype=np.float32)
    sc = 1.0 / np.sqrt(E)
    Wq = rng.uniform(-sc, sc, (H, E)).astype(np.float32)
    Wk = rng.uniform(-sc, sc, (H, E)).astype(np.float32)
    Wv = rng.uniform(-sc, sc, (H, E)).astype(np.float32)
    o = kernel(x=x, Wq=Wq, Wk=Wk, Wv=Wv)
    print(o.shape, o.dtype)
